# revision 26
# baseline (speedup 1.0000x reference)
"""ComPoM sparse-attention kernel for 8 TRN2 NeuronCores — fp8 DR v2.

Math (per batch b):
    h  = xc[b] @ Wpo.T                     (N, DE)
    a  = clip(leaky_relu(h, 0.01), -.1, 6)
    hm = (c0*S1 + c1*S2 + c2*S3) / cnt     where Sk = sum_n mask[n] * a^k
    s  = hardsigmoid(xq[b] @ Wse.T + bse)  (T, DE)
    out[b] = s @ (hm * Wag).T              (T, DIM)

Sharding over 8 cores: core c handles batch b = c//2 and
  - stage 1 (hm): DE-shard j = c%2 (1024 channels); 2-core AllGather of hm
  - stage 2 (out): T-shard j (2048 rows); outputs are disjoint.

v2 changes over the 287us baseline:
  * d-centering: s = 0.5 + d/8 with d8 = 8s-4 stored fp8.  The constant
    0.5*sum_e hm*Wag term is computed exactly (bf16 colsum matmuls + one
    K=1 broadcast matmul) so the fp8 error of the output projection only
    multiplies the small d — this lets stage 2b run fp8 DoubleRow
    (k=256/pass) instead of bf16 (k=128/pass), halving its PE time.
    NBF ei-pairs stay bf16 for error margin.
  * stage-1 poly consumers split across engines (scalar lrelu+S1 via
    accum_out, vector a^2+S2, gpsimd a^3+S3) so the PE never stalls on
    PSUM backpressure and stays at the ramped 2.4 GHz p-state.
  * the hm AllGather (~20-30us latency for 4KB) is triggered right after
    stage 1 and fully hidden under the stage-2a gate matmuls; hm is
    applied to Wag (wagf8 = Wag*hm') on vector+gpsimd during stage 2a.
"""

import numpy as np
import ml_dtypes

import concourse.bacc as bacc
import concourse.mybir as mybir
import concourse.tile as tile
from concourse.bass_utils import run_bass_kernel_spmd

B, T, N, DIM = 4, 4096, 4096, 1024
EXPAND, DEGREE = 2, 3
DE = DIM * EXPAND
N_CORES = 8
ESH = DE // 2       # stage-1 per-core channel shard
TSH = T // 2        # stage-2 per-core row shard

P = 128
NCH = 512           # free-dim chunk (one fp32 PSUM bank)
NM = 2560           # padded masked-row count for stage 1
NP1 = NM // NCH     # 5 stage-1 n-panels
ND = DIM // P       # 8 contraction d-subtiles
NDR = ND // 2       # 4 DoubleRow k-pair calls over d
NEP = ESH // P      # 8 stage-1 e-tiles
NE2 = DE // P       # 16 e-subtiles (full DE)
NTP = TSH // NCH    # 4 stage-2 t-panels
NTB = NCH // P      # 4 t-blocks per panel
NDC = DIM // NCH    # 2 output d-chunks

NBF = 1             # ei-pairs of stage 2b kept in bf16 (error margin)
NF8 = NE2 - 2 * NBF # fp8 eis in stage 2b
NG8 = NF8 // 2      # fp8 DR pair-groups
S0 = 13             # hm' = hm * 2^S0 scaling exponent
# d8' = (8d * hm') fp8  (|values| ~2, max ~40 << 240 fp8e4 max finite)
# wag8 = wag * 64 fp8   (host static)
# psum = sum d8'*wag8 = 8*2^19 * sum d*hm*wag
DS = 2.0 ** -22         # output descale (8*2^19*DS = 1)
CK1 = 2.0 ** -(S0 + 1)  # K=1 matmul const: colfull = 0.5 * colsum * 2^-S0

F32 = mybir.dt.float32
BF16 = mybir.dt.bfloat16
FP8 = mybir.dt.float8e4
OP = mybir.AluOpType
AF = mybir.ActivationFunctionType
DRMODE = mybir.MatmulPerfMode.DoubleRow

_CACHE = {}


def _build():
    nc = bacc.Bacc("TRN2", target_bir_lowering=False, debug=False,
                   enable_asserts=False, num_devices=N_CORES)

    xcT_d = nc.dram_tensor("xcT", [NP1, P, ND, NCH], FP8,
                           kind="ExternalInput").ap()
    xqT_d = nc.dram_tensor("xqT", [NTP, P, ND, NCH], FP8,
                           kind="ExternalInput").ap()
    wpo_d = nc.dram_tensor("wpo", [P, ND, ESH], FP8, kind="ExternalInput").ap()
    wse_d = nc.dram_tensor("wse", [P, ND, DE], FP8, kind="ExternalInput").ap()
    wag_d = nc.dram_tensor("wag", [P, NE2, DIM], BF16,
                           kind="ExternalInput").ap()
    wag8_d = nc.dram_tensor("wag8", [P, NF8, DIM], FP8,
                            kind="ExternalInput").ap()
    bias_d = nc.dram_tensor("bias", [P, NE2], F32, kind="ExternalInput").ap()
    coeff_d = nc.dram_tensor("coeff", [P, NEP, DEGREE], F32,
                             kind="ExternalInput").ap()
    out_d = nc.dram_tensor("out", [TSH, DIM], F32, kind="ExternalOutput").ap()

    with tile.TileContext(nc, trace_sim=False) as tc:
        with (
            tc.tile_pool(name="prep", bufs=1) as prep,
            tc.tile_pool(name="wts", bufs=1) as wts,
            tc.tile_pool(name="dram", bufs=1, space="DRAM") as dram,
        ):
            # ---- weights / constants (straight loads, host-prepped) -----
            wpo = wts.tile([P, ND, ESH], FP8, name="wpo", tag="wpo")
            wse = wts.tile([P, ND, DE], FP8, name="wse", tag="wse")
            wag = wts.tile([P, NE2, DIM], BF16, name="wag", tag="wag")
            wag8 = wts.tile([P, NF8, DIM], FP8, name="wag8", tag="wag8")
            wagb = wts.tile([P, 2 * NBF, DIM], BF16, name="wagb", tag="wagb")
            bias_sb = prep.tile([P, NE2], F32, name="bias_sb", tag="bias_sb")
            coeff_sb = prep.tile([P, NEP, DEGREE], F32, name="coeff_sb",
                                 tag="coeff_sb")
            nc.sync.dma_start(out=wpo[:], in_=wpo_d)
            nc.gpsimd.dma_start(out=bias_sb[:], in_=bias_d)
            nc.gpsimd.dma_start(out=coeff_sb[:], in_=coeff_d)

            hm_sb = prep.tile([P, NEP], F32, name="hm_sb", tag="hm_sb")

            # ---- stage 1 + stage 2a share one pool scope so the xq
            # prefetches can be triggered before the AllGather ------------
            with (
                tc.tile_pool(name="s1x", bufs=3) as s1x,
                tc.tile_pool(name="s1w", bufs=3) as s1w,
                tc.tile_pool(name="red", bufs=2) as red,
                tc.tile_pool(name="s2x", bufs=4) as s2x,
                tc.tile_pool(name="s2w", bufs=3) as s2w,
                tc.tile_pool(name="dtw", bufs=2) as dtw,
                tc.tile_pool(name="ps1", bufs=4, space="PSUM") as ps1,
                tc.tile_pool(name="ps2", bufs=3, space="PSUM") as ps2,
            ):
                S1 = [prep.tile([P, NP1], F32, name=f"S1_{ep}",
                                tag=f"S1_{ep}") for ep in range(NEP)]
                S2 = [prep.tile([P, NP1], F32, name=f"S2_{ep}",
                                tag=f"S2_{ep}") for ep in range(NEP)]
                S3 = [prep.tile([P, NP1], F32, name=f"S3_{ep}",
                                tag=f"S3_{ep}") for ep in range(NEP)]

                def load_xc(pi):
                    t = s1x.tile([P, ND, NCH], FP8, name="xc", tag="xc")
                    nc.gpsimd.dma_start(out=t[:], in_=xcT_d[pi])
                    return t

                xc_next = load_xc(0)
                nc.sync.dma_start(out=wse[:], in_=wse_d)
                nc.sync.dma_start(out=wag[:], in_=wag_d)
                nc.sync.dma_start(out=wag8[:], in_=wag8_d)
                # prefetch all xq panels now: 4 pool bufs, so every DMA can
                # run during stage 1, and the triggers sit ahead of the
                # (blocking) AllGather on the gpsimd queue
                xqs = []
                for tp in range(NTP):
                    t = s2x.tile([P, ND, NCH], FP8, name="xq", tag="xq")
                    nc.gpsimd.dma_start(out=t[:], in_=xqT_d[tp])
                    xqs.append(t)

                # ---- stage 1: h = xc @ WpoT (fp8 DR), poly + sums -------
                # consumers: scalar (lrelu+S1 via accum), vector (a^2+S2,
                # a^3+S3, bf16 at 2x DVE rate)
                for pi in range(NP1):
                    xc = xc_next
                    if pi + 1 < NP1:
                        xc_next = load_xc(pi + 1)
                    for ep in range(NEP):
                        ps = ps1.tile([P, NCH], F32, name="h", tag="h")
                        for kk in range(NDR):
                            nc.tensor.matmul(
                                ps[:],
                                lhsT=wpo[:, 2 * kk:2 * kk + 2,
                                         ep * P:(ep + 1) * P],
                                rhs=xc[:, 2 * kk:2 * kk + 2, :],
                                start=(kk == 0), stop=(kk == NDR - 1),
                                perf_mode=DRMODE)
                        a = s1w.tile([P, NCH], BF16, name="a", tag="a")
                        a2 = s1w.tile([P, NCH], BF16, name="a2", tag="a2")
                        a3 = s1w.tile([P, NCH], BF16, name="a3", tag="a3")
                        nc.scalar.activation(out=a[:], in_=ps[:],
                                             func=AF.Lrelu, alpha=0.01,
                                             accum_out=S1[ep][:, pi:pi + 1])
                        nc.vector.scalar_tensor_tensor(
                            out=a2[:], in0=a[:], scalar=1.0, in1=a[:],
                            op0=OP.mult, op1=OP.mult,
                            accum_out=S2[ep][:, pi:pi + 1])
                        nc.vector.scalar_tensor_tensor(
                            out=a3[:], in0=a2[:], scalar=1.0, in1=a[:],
                            op0=OP.mult, op1=OP.mult,
                            accum_out=S3[ep][:, pi:pi + 1])

                # hm'_shard[e] = c0*S1 + c1*S2 + c2*S3  (coeff pre *2^S0/cnt)
                for ep in range(NEP):
                    s1r = red.tile([P, 1], F32, name="s1r", tag="s1r")
                    s2r = red.tile([P, 1], F32, name="s2r", tag="s2r")
                    s3r = red.tile([P, 1], F32, name="s3r", tag="s3r")
                    nc.vector.reduce_sum(out=s1r[:], in_=S1[ep][:],
                                         axis=mybir.AxisListType.X)
                    nc.vector.reduce_sum(out=s2r[:], in_=S2[ep][:],
                                         axis=mybir.AxisListType.X)
                    nc.vector.reduce_sum(out=s3r[:], in_=S3[ep][:],
                                         axis=mybir.AxisListType.X)
                    u1 = red.tile([P, 1], F32, name="u1", tag="u1")
                    u2 = red.tile([P, 1], F32, name="u2", tag="u2")
                    c0 = coeff_sb[:, ep, 0:1]
                    c1 = coeff_sb[:, ep, 1:2]
                    c2 = coeff_sb[:, ep, 2:3]
                    nc.vector.tensor_scalar(out=u1[:], in0=s1r[:], scalar1=c0,
                                            scalar2=None, op0=OP.mult)
                    nc.vector.scalar_tensor_tensor(
                        out=u2[:], in0=s2r[:], scalar=c1, in1=u1[:],
                        op0=OP.mult, op1=OP.add)
                    nc.vector.scalar_tensor_tensor(
                        out=hm_sb[:, ep:ep + 1], in0=s3r[:], scalar=c2,
                        in1=u2[:], op0=OP.mult, op1=OP.add)

                # hm' AllGather across batch pairs (hidden under stage 2a)
                hm_dram = dram.tile([ESH], F32, name="hm_dram",
                                    tag="hm_dram")
                hmall_dram = dram.tile([DE], F32, name="hmall_dram",
                                       tag="hmall_dram")
                nc.sync.dma_start(out=hm_dram.rearrange("(a p) -> p a", p=P),
                                  in_=hm_sb[:])
                nc.gpsimd.collective_compute(
                    "AllGather", OP.bypass,
                    replica_groups=[[0, 1], [2, 3], [4, 5], [6, 7]],
                    ins=[hm_dram.opt()], outs=[hmall_dram.opt()])
                hmall_sb = prep.tile([P, NE2], F32, name="hmall_sb",
                                     tag="hmall_sb")
                nc.sync.dma_start(out=hmall_sb[:],
                                  in_=hmall_dram.rearrange("(a p) -> p a",
                                                           p=P))

                # ---- stage 2a: z = xq @ WseT (fp8 DR) -------------------
                # d8' = (8d * hm') fp8 via scalar Copy (idle engine);
                # affine+clip on DVE in bf16 (fast)
                dT8 = [prep.tile([P, NF8, NCH], FP8, name=f"dT8_{tp}",
                                 tag=f"dT8_{tp}") for tp in range(NTP)]
                dTb = [prep.tile([P, 2 * NBF, NCH], BF16, name=f"dTb_{tp}",
                                 tag=f"dTb_{tp}") for tp in range(NTP)]
                hmall_b = prep.tile([P, NE2], BF16, name="hmall_b",
                                    tag="hmall_b")

                for tp in range(NTP):
                    xq = xqs[tp]
                    # rotating bf16 staging for 8d; lives only until the
                    # panel's fp8 copies have run (1-2 panels)
                    dT16 = dtw.tile([P, NF8, NCH], BF16, name="dT16",
                                    tag="dT16")
                    for ei in range(NE2):
                        ps = ps2.tile([P, NCH], F32, name="z", tag="z")
                        for kk in range(NDR):
                            nc.tensor.matmul(
                                ps[:],
                                lhsT=wse[:, 2 * kk:2 * kk + 2,
                                         ei * P:(ei + 1) * P],
                                rhs=xq[:, 2 * kk:2 * kk + 2, :],
                                start=(kk == 0), stop=(kk == NDR - 1),
                                perf_mode=DRMODE)
                        # y = z*(8/6) + (8*(bse/6+0.5) - 4); 8d = clip(y,+-4)
                        tmp = s2w.tile([P, NCH], BF16, name="tmp", tag="tmp")
                        nc.vector.tensor_scalar(
                            out=tmp[:], in0=ps[:], scalar1=8.0 / 6.0,
                            scalar2=bias_sb[:, ei:ei + 1],
                            op0=OP.mult, op1=OP.add)
                        dst = (dT16[:, ei, :] if ei < NF8
                               else dTb[tp][:, ei - NF8, :])
                        nc.vector.tensor_scalar(out=dst, in0=tmp[:],
                                                scalar1=4.0, scalar2=-4.0,
                                                op0=OP.min, op1=OP.max)
                    if tp == 1:
                        # hmall has landed by now (AG under panels 0-1)
                        nc.vector.tensor_copy(out=hmall_b[:],
                                              in_=hmall_sb[:])
                        for k in range(2 * NBF):
                            # wagb = wag * hm' * 64 (matches wag8's 64)
                            nc.vector.tensor_scalar(
                                out=wagb[:, k, :], in0=wag[:, NF8 + k, :],
                                scalar1=hmall_sb[:, NF8 + k:NF8 + k + 1],
                                scalar2=64.0, op0=OP.mult, op1=OP.mult)
                    # d8' fp8 conversions ride the idle scalar engine; they
                    # wait on hmall via semaphores, not queue order
                    for ei in range(NF8):
                        nc.scalar.activation(
                            out=dT8[tp][:, ei, :], in_=dT16[:, ei, :],
                            func=AF.Copy, scale=hmall_sb[:, ei:ei + 1])

            # ---- colsum + colfull: 0.5 * sum_e hm*wag, broadcast --------
            with (
                tc.tile_pool(name="psc", bufs=2, space="PSUM") as psc,
            ):
                colsum_sb = prep.tile([1, NDC, NCH], BF16, name="colsum_sb",
                                      tag="colsum_sb")
                c16 = prep.tile([1, P], BF16, name="c16", tag="c16")
                nc.gpsimd.memset(c16[:], CK1)
                colfull = prep.tile([P, NDC, NCH], F32, name="colfull",
                                    tag="colfull")
                for dc in range(NDC):
                    pc = psc.tile([1, NCH], F32, name="pc", tag="pc")
                    for ei in range(NE2):
                        nc.tensor.matmul(
                            pc[:], lhsT=hmall_b[:, ei:ei + 1],
                            rhs=wag[:, ei, dc * NCH:(dc + 1) * NCH],
                            start=(ei == 0), stop=(ei == NE2 - 1))
                    nc.vector.tensor_copy(out=colsum_sb[:, dc, :], in_=pc[:])
                for dc in range(NDC):
                    pf = psc.tile([P, NCH], F32, name="pf", tag="pf")
                    nc.tensor.matmul(pf[:], lhsT=c16[:],
                                     rhs=colsum_sb[:, dc, :],
                                     start=True, stop=True)
                    nc.vector.tensor_copy(out=colfull[:, dc, :], in_=pf[:])

            # ---- stage 2b: out = colfull + 2^-(S0+3) * d8 @ wagf8.T -----
            with (
                tc.tile_pool(name="s2o", bufs=2) as s2o,
                tc.tile_pool(name="ps3", bufs=4, space="PSUM") as ps3,
            ):
                for tp in range(NTP):
                    for tb in range(NTB):
                        pso = [ps3.tile([P, NCH], F32, name=f"o{dc}",
                                        tag=f"o{dc}") for dc in range(NDC)]
                        for dc in range(NDC):
                            for g in range(NG8):
                                nc.tensor.matmul(
                                    pso[dc][:],
                                    lhsT=dT8[tp][:, 2 * g:2 * g + 2,
                                                 tb * P:(tb + 1) * P],
                                    rhs=wag8[:, 2 * g:2 * g + 2,
                                             dc * NCH:(dc + 1) * NCH],
                                    start=(g == 0), stop=False,
                                    perf_mode=DRMODE)
                            for k in range(2 * NBF):
                                nc.tensor.matmul(
                                    pso[dc][:],
                                    lhsT=dTb[tp][:, k, tb * P:(tb + 1) * P],
                                    rhs=wagb[:, k, dc * NCH:(dc + 1) * NCH],
                                    start=False, stop=(k == 2 * NBF - 1))
                        ob = s2o.tile([P, DIM], F32, name="ob", tag="ob")
                        for dc in range(NDC):
                            nc.vector.scalar_tensor_tensor(
                                out=ob[:, dc * NCH:(dc + 1) * NCH],
                                in0=pso[dc][:], scalar=DS,
                                in1=colfull[:, dc, :],
                                op0=OP.mult, op1=OP.add)
                        r0 = tp * NCH + tb * P
                        nc.scalar.dma_start(out=out_d[r0:r0 + P, :],
                                            in_=ob[:])

    nc.compile()
    return nc


def _get_nc():
    if "nc" not in _CACHE:
        _CACHE["nc"] = _build()
    return _CACHE["nc"]


F8NP = ml_dtypes.float8_e4m3
BFNP = ml_dtypes.bfloat16


def _pack_kdim(arr_kf, dt):
    """[K, F] (contraction-major) -> [P, K//P, F] SBUF layout, cast."""
    K, F = arr_kf.shape
    return np.ascontiguousarray(
        arr_kf.reshape(K // P, P, F).transpose(1, 0, 2)).astype(dt)


def _pack_panels(arr_kf, nch, dt):
    """[K, F] -> [F//nch, P, K//P, nch] (per-panel contiguous), cast."""
    K, F = arr_kf.shape
    a = arr_kf.reshape(K // P, P, F // nch, nch).transpose(2, 1, 0, 3)
    return np.ascontiguousarray(a).astype(dt)


def kernel(xq, xc, mask, Wpo, Wse, bse, coeff, Wag, _trace=False):
    nc = _get_nc()
    xq = np.asarray(xq, np.float32)
    xc = np.asarray(xc, np.float32)
    mask = np.asarray(mask, np.int32)
    Wpo = np.asarray(Wpo, np.float32)
    Wse = np.asarray(Wse, np.float32)
    bse = np.asarray(bse, np.float32)
    coeff = np.asarray(coeff, np.float32)
    Wag = np.asarray(Wag, np.float32)

    # host prep: transposes + fp8/bf16 casts in exact SBUF layouts
    wpo8 = [_pack_kdim(np.ascontiguousarray(
        Wpo[j * ESH:(j + 1) * ESH].T), F8NP) for j in range(2)]
    wse8 = _pack_kdim(np.ascontiguousarray(Wse.T), F8NP)
    wag16 = _pack_kdim(np.ascontiguousarray(Wag.T), BFNP)
    # y = z*(8/6) + bias, 8d = clip(y, -4, 4)  (== min(relu(y+4),8)-4)
    biasp = np.ascontiguousarray(
        (8.0 * (bse / 6.0 + 0.5) - 4.0).reshape(NE2, P).T).astype(np.float32)
    wag8 = (wag16[:, :NF8, :].astype(np.float32) * 64.0).astype(F8NP)

    xcT8 = []     # per batch: [NP1, P, ND, NCH] fp8 of masked+padded xc.T
    rcnt = []
    for b in range(B):
        idx = np.nonzero(mask[b])[0]
        rcnt.append(2.0 ** S0 / len(idx))
        Xg = np.zeros((NM, DIM), np.float32)
        Xg[:len(idx)] = xc[b][idx]
        xcT8.append(_pack_panels(np.ascontiguousarray(Xg.T), NCH, F8NP))

    in_maps = []
    for c in range(N_CORES):
        b, j = c // 2, c % 2
        xqT8 = _pack_panels(np.ascontiguousarray(
            xq[b, j * TSH:(j + 1) * TSH].T), NCH, F8NP)
        cj = (coeff[j * ESH:(j + 1) * ESH] * rcnt[b]).reshape(
            NEP, P, DEGREE).transpose(1, 0, 2)
        in_maps.append({
            "xcT": xcT8[b],
            "xqT": xqT8,
            "wpo": wpo8[j],
            "wse": wse8,
            "wag": wag16,
            "wag8": wag8,
            "bias": biasp,
            "coeff": np.ascontiguousarray(cj).astype(np.float32),
        })
    res = run_bass_kernel_spmd(nc, in_maps, list(range(N_CORES)), trace=_trace)
    out = np.empty((B, T, DIM), np.float32)
    for c in range(N_CORES):
        b, j = c // 2, c % 2
        out[b, j * TSH:(j + 1) * TSH] = res.results[c]["out"]
    if _trace:
        _CACHE["last_result"] = res
    return out
